# revision 2
# baseline (speedup 1.0000x reference)
"""Trainium2 Bass kernel for KernelAttention (gaussian-kernel multi-head attention).

Math (per batch b):
  d2[q,k]   = |q_pos[q] - k_pos[k]|^2   (computed as -d2 via one K=5 augmented matmul)
  s_h[k,q]  = exp(-c_h * d2),  c_h = 1/lengthscale_h^2   (masked keys contribute 0)
  att_h[q,v]= sum_k s_h[k,q] * V[k,h,v] / (sum_k s_h[k,q]*unmasked[k] + 1e-5)
  out[q,o]  = sum_{h,v} att_h[q,v] * w_out[o, h*64+v]

Sharding: 8 cores = (batch b in 0..3) x (query half in 0..1); each core owns
[1024 q, 2048 k]. All inputs host-prepped per core; outputs gathered on host.

Device-side layout is score-transposed: s_h is [k, q] so the attend matmul
(lhsT = values, rhs = scores) contracts k on the PE partition dim directly.
Masking + normalization are folded in: masked V rows are zeroed on the host and
a per-head ones-column (also mask-zeroed) produces the normalizer as psum row 64.
Normalization is deferred past the attend; the reciprocal is computed via
ACT Ln/Exp and broadcast across partitions with a tiny K=8 selection matmul.
Two heads (c=4, c=0.04) are derived from their 4x-smaller neighbors by two DVE
squarings, offloading exp work from the scalar engine.
"""

import numpy as np
from contextlib import ExitStack

B, LQ, LK, DPOS = 4, 2048, 2048, 3
H, V, OUTD = 8, 64, 512
QS = LQ // 2          # q rows per core
KT = LK // 128        # k tiles
V1 = V + 1            # value cols + ones col
NCORES = 8

# head processing order: chain sources immediately before their derived heads
ORDER = [3, 2, 6, 5, 0, 1, 4, 7]
DERIVED = {2: 3, 5: 6}  # derived_head -> source_head, s_d = s_src ** 4

_cache = {}


def _build(cv, use_chains):
    key = (tuple(cv), use_chains)
    if key in _cache:
        return _cache[key]
    import concourse.bacc as bacc
    import concourse.tile as tile
    from concourse import mybir

    f32 = mybir.dt.float32
    bf16 = mybir.dt.bfloat16
    AF = mybir.ActivationFunctionType

    nc = bacc.Bacc("TRN2", target_bir_lowering=False, debug=False,
                   num_devices=NCORES)
    ka = nc.dram_tensor("ka", [5, LK], f32, kind="ExternalInput").ap()
    qa = nc.dram_tensor("qa", [5, QS], f32, kind="ExternalInput").ap()
    vp = nc.dram_tensor("vp", [128, KT, H * V1], bf16, kind="ExternalInput").ap()
    wt = nc.dram_tensor("wt", [128, 4, OUTD], bf16, kind="ExternalInput").ap()
    sel8 = nc.dram_tensor("sel8", [8, 4, 128], f32, kind="ExternalInput").ap()
    outT = nc.dram_tensor("outT", [OUTD, QS], f32, kind="ExternalOutput").ap()

    with tile.TileContext(nc) as tc, ExitStack() as ctx:
        const = ctx.enter_context(tc.tile_pool(name="const", bufs=1))
        spool = ctx.enter_context(tc.tile_pool(name="spool", bufs=3))
        stage = ctx.enter_context(tc.tile_pool(name="stage", bufs=2))
        obuf = ctx.enter_context(tc.tile_pool(name="obuf", bufs=2))
        psp = ctx.enter_context(tc.tile_pool(name="psum", bufs=3, space="PSUM"))

        ka_sb = const.tile([5, LK], f32)
        nc.sync.dma_start(out=ka_sb[:], in_=ka)
        qa_sb = const.tile([5, QS], f32)
        nc.sync.dma_start(out=qa_sb[:], in_=qa)
        vp_sb = const.tile([128, KT, H * V1], bf16)
        nc.sync.dma_start(out=vp_sb[:], in_=vp)
        wt_sb = const.tile([128, 4, OUTD], bf16)
        nc.sync.dma_start(out=wt_sb[:], in_=wt)
        sel8_sb = const.tile([8, 4, 128], f32)
        nc.sync.dma_start(out=sel8_sb[:], in_=sel8)

        # Phase A: m = -d2 in [k, q] layout, evacuated to bf16 SBUF
        m_all = const.tile([128, KT, QS], bf16)
        for kt in range(KT):
            d2 = psp.tile([128, QS], f32, tag="ps")
            for qc in range(2):
                s5 = slice(qc * 512, (qc + 1) * 512)
                nc.tensor.matmul(d2[:, s5],
                                 lhsT=ka_sb[:, kt * 128:(kt + 1) * 128],
                                 rhs=qa_sb[:, s5], start=True, stop=True)
            nc.vector.tensor_copy(out=m_all[:, kt, :], in_=d2[:])

        flat = const.tile([128, 4, QS], bf16)
        norms = const.tile([8, QS], f32)
        s_tiles = {}
        for h in ORDER:
            if use_chains and h in DERIVED:
                src = s_tiles[DERIVED[h]]
                s = spool.tile([128, KT, QS], bf16, tag="s")
                nc.vector.tensor_mul(s[:], src[:], src[:])
                nc.vector.tensor_mul(s[:], s[:], s[:])
            else:
                s = spool.tile([128, KT, QS], bf16, tag="s")
                nc.scalar.activation(out=s[:], in_=m_all[:], func=AF.Exp,
                                     scale=float(cv[h]))
            s_tiles[h] = s

            att = psp.tile([V1, QS], f32, tag="ps")
            for kt in range(KT):
                for qc in range(2):
                    s5 = slice(qc * 512, (qc + 1) * 512)
                    nc.tensor.matmul(att[:, s5],
                                     lhsT=vp_sb[:, kt, h * V1:(h + 1) * V1],
                                     rhs=s[:, kt, s5],
                                     start=(kt == 0), stop=(kt == KT - 1))
            r0 = (h % 2) * 64
            nc.vector.tensor_copy(out=flat[r0:r0 + 64, h // 2, :],
                                  in_=att[0:64, :])
            stg = stage.tile([V1, QS], f32, tag="stg")
            nc.vector.tensor_copy(out=stg[64:65, :], in_=att[64:65, :])
            nc.sync.dma_start(out=norms[h:h + 1, :], in_=stg[64:65, :])

        # r = 1/(norm + 1e-5) via exp(-ln(norm + 1e-5)); Ln+Exp share a table set
        eps_t = const.tile([8, 1], f32)
        nc.vector.memset(eps_t[:], 1e-5)
        lnn = const.tile([8, QS], f32)
        nc.scalar.activation(out=lnn[:], in_=norms[:], func=AF.Ln, bias=eps_t[:])
        r_all = const.tile([8, QS], f32)
        nc.scalar.activation(out=r_all[:], in_=lnn[:], func=AF.Exp, scale=-1.0)

        # broadcast r across partitions (head pair j -> 128 rows) and normalize
        for j in range(4):
            rb = psp.tile([128, QS], f32, tag="ps")
            for qc in range(2):
                s5 = slice(qc * 512, (qc + 1) * 512)
                nc.tensor.matmul(rb[:, s5], lhsT=sel8_sb[:, j, :],
                                 rhs=r_all[:, s5], start=True, stop=True)
            nc.vector.tensor_mul(flat[:, j, :], flat[:, j, :], rb[:])

        # out projection: outT[o, q] = sum_hv wt[hv, o] * flat[hv, q]
        for ot in range(4):
            po = psp.tile([128, QS], f32, tag="ps")
            for j in range(4):
                for qc in range(2):
                    s5 = slice(qc * 512, (qc + 1) * 512)
                    nc.tensor.matmul(po[:, s5],
                                     lhsT=wt_sb[:, j, ot * 128:(ot + 1) * 128],
                                     rhs=flat[:, j, s5],
                                     start=(j == 0), stop=(j == 3))
            ob = obuf.tile([128, QS], f32, tag="ob")
            nc.scalar.copy(out=ob[:], in_=po[:])
            nc.sync.dma_start(out=outT[ot * 128:(ot + 1) * 128, :], in_=ob[:])

    nc.compile()
    _cache[key] = nc
    return nc


def _prep_core(qp, kp, vals, mask, w_out, bf16):
    q2 = (qp * qp).sum(-1)
    one_q = np.ones(QS, np.float32)
    qa = np.stack([2 * qp[:, 0], 2 * qp[:, 1], 2 * qp[:, 2], -one_q, -q2]) \
        .astype(np.float32)
    k2 = (kp * kp).sum(-1)
    one_k = np.ones(LK, np.float32)
    ka = np.stack([kp[:, 0], kp[:, 1], kp[:, 2], k2, one_k]).astype(np.float32)
    vv = np.concatenate([vals, np.ones((LK, H, 1), np.float32)], axis=-1)
    vv = vv.copy()
    vv[mask] = 0.0
    vp = vv.reshape(KT, 128, H * V1).transpose(1, 0, 2).astype(bf16)
    wt = np.ascontiguousarray(w_out.T).reshape(4, 128, OUTD) \
        .transpose(1, 0, 2).astype(bf16)
    sel8 = np.zeros((8, 4, 128), np.float32)
    for j in range(4):
        sel8[2 * j, j, :64] = 1.0
        sel8[2 * j + 1, j, 64:] = 1.0
    return {"ka": ka, "qa": np.ascontiguousarray(qa), "vp": np.ascontiguousarray(vp),
            "wt": np.ascontiguousarray(wt), "sel8": sel8}


def kernel(query_positions, key_positions, values, masked_elements,
           lengthscales, w_out, _want_trace=False):
    import ml_dtypes
    from concourse.bass_utils import run_bass_kernel_spmd

    bf16 = ml_dtypes.bfloat16
    qp = np.asarray(query_positions, np.float32)
    kp = np.asarray(key_positions, np.float32)
    vals = np.asarray(values, np.float32)
    mask = np.asarray(masked_elements).astype(bool)
    ls = np.asarray(lengthscales, np.float32)
    w = np.asarray(w_out, np.float32)

    cv = (1.0 / (ls.astype(np.float64) ** 2)).astype(np.float32)
    use_chains = all(
        np.float32(cv[d]) == np.float32(4.0) * np.float32(cv[s])
        for d, s in DERIVED.items())
    nc = _build(tuple(float(x) for x in cv), use_chains)

    in_maps = []
    for c in range(NCORES):
        b, hf = c // 2, c % 2
        in_maps.append(_prep_core(qp[b, hf * QS:(hf + 1) * QS], kp[b],
                                  vals[b], mask[b], w, bf16))
    res = run_bass_kernel_spmd(nc, in_maps, core_ids=list(range(NCORES)),
                               trace=_want_trace)
    out = np.empty((B, LQ, OUTD), np.float32)
    for c in range(NCORES):
        b, hf = c // 2, c % 2
        out[b, hf * QS:(hf + 1) * QS, :] = res.results[c]["outT"].T
    if _want_trace:
        return out, res
    return out


# revision 7
# speedup vs baseline: 1.2485x; 1.2485x over previous
"""Trainium2 Bass kernel for KernelAttention (gaussian-kernel multi-head attention).

Math (per batch b):
  d2[q,k]   = |q_pos[q] - k_pos[k]|^2   (computed as -d2 via one K=5 augmented matmul)
  s_h[k,q]  = exp(-c_h * d2),  c_h = 1/lengthscale_h^2   (masked keys contribute 0)
  att_h[q,v]= sum_k s_h[k,q] * V[k,h,v] / (sum_k s_h[k,q]*unmasked[k] + 1e-5)
  out[q,o]  = sum_{h,v} att_h[q,v] * w_out[o, h*64+v]

Sharding: 8 cores = (batch b in 0..3) x (query half in 0..1); each core owns
[1024 q, 2048 k]. All inputs host-prepped per core; outputs gathered on host.

Device-side layout is score-transposed: s_h is [k, q] so the attend matmul
(lhsT = values, rhs = scores) contracts k on the PE partition dim directly.
Masking + normalization are folded in: masked V rows are zeroed on the host and
a per-head ones-column (also mask-zeroed) produces the normalizer as psum row 64.
Normalization is deferred past the attend; the reciprocal is computed via
ACT Ln/Exp and broadcast across partitions with a tiny K=8 selection matmul.
Two heads (c=4, c=0.04) are derived from their 4x-smaller neighbors by two DVE
squarings, offloading exp work from the scalar engine.
"""

import numpy as np
from contextlib import ExitStack

B, LQ, LK, DPOS = 4, 2048, 2048, 3
H, V, OUTD = 8, 64, 512
QS = LQ // 2          # q rows per core
KT = LK // 128        # k tiles
V1 = V + 1            # value cols + ones col
NCORES = 8

# head processing order: chain sources immediately before their derived heads
ORDER = [3, 2, 6, 5, 0, 1, 4, 7]
DERIVED = {2: 3, 5: 6}  # derived_head -> source_head, s_d = s_src ** 4

_cache = {}


def _build(cv, use_chains):
    key = (tuple(cv), use_chains)
    if key in _cache:
        return _cache[key]
    import concourse.bacc as bacc
    import concourse.tile as tile
    from concourse import mybir

    f32 = mybir.dt.float32
    bf16 = mybir.dt.bfloat16
    AF = mybir.ActivationFunctionType

    nc = bacc.Bacc("TRN2", target_bir_lowering=False, debug=False,
                   num_devices=NCORES)
    # ka/qa carry a hi/lo bf16 split of the K=5 augmented distance operands:
    # rows [hi(5); lo(5); hi(5)] x [hi(5); hi(5); lo(5)] so the single bf16
    # matmul accumulates hi*hi + lo*hi + hi*lo in fp32 PSUM (lo*lo ~2^-16 is
    # dropped). This avoids fp32 LOW_HIGH double-pass matmuls entirely.
    ka = nc.dram_tensor("ka", [15, LK], bf16, kind="ExternalInput").ap()
    qa = nc.dram_tensor("qa", [15, QS], bf16, kind="ExternalInput").ap()
    vp = nc.dram_tensor("vp", [128, KT, H * V1], bf16, kind="ExternalInput").ap()
    wt = nc.dram_tensor("wt", [128, 4, OUTD], bf16, kind="ExternalInput").ap()
    sel8 = nc.dram_tensor("sel8", [8, 4, 128], bf16, kind="ExternalInput").ap()
    outT = nc.dram_tensor("outT", [OUTD, QS], f32, kind="ExternalOutput").ap()

    with tile.TileContext(nc) as tc, ExitStack() as ctx:
        const = ctx.enter_context(tc.tile_pool(name="const", bufs=1))
        spool = ctx.enter_context(tc.tile_pool(name="spool", bufs=3))
        stage = ctx.enter_context(tc.tile_pool(name="stage", bufs=2))
        obuf = ctx.enter_context(tc.tile_pool(name="obuf", bufs=2))
        psp = ctx.enter_context(tc.tile_pool(name="psum", bufs=3, space="PSUM"))

        ka_sb = const.tile([15, LK], bf16)
        nc.sync.dma_start(out=ka_sb[:], in_=ka)
        qa_sb = const.tile([15, QS], bf16)
        nc.sync.dma_start(out=qa_sb[:], in_=qa)
        vp_sb = const.tile([128, KT, H * V1], bf16)
        nc.sync.dma_start(out=vp_sb[:], in_=vp)
        wt_sb = const.tile([128, 4, OUTD], bf16)
        nc.sync.dma_start(out=wt_sb[:], in_=wt)
        sel8_sb = const.tile([8, 4, 128], bf16)
        nc.sync.dma_start(out=sel8_sb[:], in_=sel8)

        # Phase A: m = -d2 in [k, q] layout, evacuated to bf16 SBUF
        m_all = const.tile([128, KT, QS], bf16)
        for kt in range(KT):
            d2 = psp.tile([128, QS], f32, tag="ps")
            for qc in range(2):
                s5 = slice(qc * 512, (qc + 1) * 512)
                nc.tensor.matmul(d2[:, s5],
                                 lhsT=ka_sb[:, kt * 128:(kt + 1) * 128],
                                 rhs=qa_sb[:, s5], start=True, stop=True)
            nc.vector.tensor_copy(out=m_all[:, kt, :], in_=d2[:])

        flat = const.tile([128, 4, QS], bf16)
        norms = const.tile([8, QS], f32)
        s_tiles = {}
        for h in ORDER:
            if use_chains and h in DERIVED:
                src = s_tiles[DERIVED[h]]
                s = spool.tile([128, KT, QS], bf16, tag="s")
                nc.vector.tensor_mul(s[:], src[:], src[:])
                nc.vector.tensor_mul(s[:], s[:], s[:])
            else:
                s = spool.tile([128, KT, QS], bf16, tag="s")
                nc.scalar.activation(out=s[:], in_=m_all[:], func=AF.Exp,
                                     scale=float(cv[h]))
            s_tiles[h] = s

            att = psp.tile([V1, QS], f32, tag="ps")
            for kt in range(KT):
                for qc in range(2):
                    s5 = slice(qc * 512, (qc + 1) * 512)
                    nc.tensor.matmul(att[:, s5],
                                     lhsT=vp_sb[:, kt, h * V1:(h + 1) * V1],
                                     rhs=s[:, kt, s5],
                                     start=(kt == 0), stop=(kt == KT - 1))
            r0 = (h % 2) * 64
            nc.vector.tensor_copy(out=flat[r0:r0 + 64, h // 2, :],
                                  in_=att[0:64, :])
            stg = stage.tile([V1, QS], f32, tag="stg")
            nc.vector.tensor_copy(out=stg[64:65, :], in_=att[64:65, :])
            nc.sync.dma_start(out=norms[h:h + 1, :], in_=stg[64:65, :])

        # r = 1/(norm + 1e-5) via exp(-ln(norm + 1e-5)); Ln+Exp share a table set
        eps_t = const.tile([8, 1], f32)
        nc.vector.memset(eps_t[:], 1e-5)
        lnn = const.tile([8, QS], f32)
        nc.scalar.activation(out=lnn[:], in_=norms[:], func=AF.Ln, bias=eps_t[:])
        r_all = const.tile([8, QS], f32)
        nc.scalar.activation(out=r_all[:], in_=lnn[:], func=AF.Exp, scale=-1.0)
        # hi/lo bf16 split of r so the broadcast matmul stays bf16-exact
        r_hi = const.tile([8, QS], bf16)
        nc.vector.tensor_copy(out=r_hi[:], in_=r_all[:])
        r_lo = const.tile([8, QS], bf16)
        nc.vector.tensor_sub(r_lo[:], r_all[:], r_hi[:])

        # broadcast r across partitions (head pair j -> 128 rows) and normalize
        for j in range(4):
            rb = psp.tile([128, QS], f32, tag="ps")
            for qc in range(2):
                s5 = slice(qc * 512, (qc + 1) * 512)
                nc.tensor.matmul(rb[:, s5], lhsT=sel8_sb[:, j, :],
                                 rhs=r_hi[:, s5], start=True, stop=False)
                nc.tensor.matmul(rb[:, s5], lhsT=sel8_sb[:, j, :],
                                 rhs=r_lo[:, s5], start=False, stop=True)
            nc.vector.tensor_mul(flat[:, j, :], flat[:, j, :], rb[:])

        # out projection: outT[o, q] = sum_hv wt[hv, o] * flat[hv, q]
        for ot in range(4):
            po = psp.tile([128, QS], f32, tag="ps")
            for j in range(4):
                for qc in range(2):
                    s5 = slice(qc * 512, (qc + 1) * 512)
                    nc.tensor.matmul(po[:, s5],
                                     lhsT=wt_sb[:, j, ot * 128:(ot + 1) * 128],
                                     rhs=flat[:, j, s5],
                                     start=(j == 0), stop=(j == 3))
            ob = obuf.tile([128, QS], f32, tag="ob")
            nc.scalar.copy(out=ob[:], in_=po[:])
            nc.sync.dma_start(out=outT[ot * 128:(ot + 1) * 128, :], in_=ob[:])

    nc.compile()
    _cache[key] = nc
    return nc


def _hilo(x, bf16):
    hi = x.astype(bf16)
    lo = (x - hi.astype(np.float32)).astype(bf16)
    return hi, lo


def _prep_core(qp, kp, vals, mask, w_out, bf16):
    q2 = (qp * qp).sum(-1)
    one_q = np.ones(QS, np.float32)
    qa5 = np.stack([2 * qp[:, 0], 2 * qp[:, 1], 2 * qp[:, 2], -one_q, -q2]) \
        .astype(np.float32)
    k2 = (kp * kp).sum(-1)
    one_k = np.ones(LK, np.float32)
    ka5 = np.stack([kp[:, 0], kp[:, 1], kp[:, 2], k2, one_k]).astype(np.float32)
    ka_hi, ka_lo = _hilo(ka5, bf16)
    qa_hi, qa_lo = _hilo(qa5, bf16)
    ka = np.concatenate([ka_hi, ka_lo, ka_hi])   # [15, LK]
    qa = np.concatenate([qa_hi, qa_hi, qa_lo])   # [15, QS]
    vv = np.concatenate([vals, np.ones((LK, H, 1), np.float32)], axis=-1)
    vv = vv.copy()
    vv[mask] = 0.0
    vp = vv.reshape(KT, 128, H * V1).transpose(1, 0, 2).astype(bf16)
    wt = np.ascontiguousarray(w_out.T).reshape(4, 128, OUTD) \
        .transpose(1, 0, 2).astype(bf16)
    sel8 = np.zeros((8, 4, 128), np.float32)
    for j in range(4):
        sel8[2 * j, j, :64] = 1.0
        sel8[2 * j + 1, j, 64:] = 1.0
    return {"ka": np.ascontiguousarray(ka), "qa": np.ascontiguousarray(qa),
            "vp": np.ascontiguousarray(vp), "wt": np.ascontiguousarray(wt),
            "sel8": sel8.astype(bf16)}


def kernel(query_positions, key_positions, values, masked_elements,
           lengthscales, w_out, _want_trace=False):
    import ml_dtypes
    from concourse.bass_utils import run_bass_kernel_spmd

    bf16 = ml_dtypes.bfloat16
    qp = np.asarray(query_positions, np.float32)
    kp = np.asarray(key_positions, np.float32)
    vals = np.asarray(values, np.float32)
    mask = np.asarray(masked_elements).astype(bool)
    ls = np.asarray(lengthscales, np.float32)
    w = np.asarray(w_out, np.float32)

    cv = (1.0 / (ls.astype(np.float64) ** 2)).astype(np.float32)
    use_chains = all(
        np.float32(cv[d]) == np.float32(4.0) * np.float32(cv[s])
        for d, s in DERIVED.items())
    nc = _build(tuple(float(x) for x in cv), use_chains)

    in_maps = []
    for c in range(NCORES):
        b, hf = c // 2, c % 2
        in_maps.append(_prep_core(qp[b, hf * QS:(hf + 1) * QS], kp[b],
                                  vals[b], mask[b], w, bf16))
    res = run_bass_kernel_spmd(nc, in_maps, core_ids=list(range(NCORES)),
                               trace=_want_trace)
    out = np.empty((B, LQ, OUTD), np.float32)
    for c in range(NCORES):
        b, hf = c // 2, c % 2
        out[b, hf * QS:(hf + 1) * QS, :] = res.results[c]["outT"].T
    if _want_trace:
        return out, res
    return out


# revision 10
# speedup vs baseline: 1.4526x; 1.1635x over previous
"""Trainium2 Bass kernel for KernelAttention (gaussian-kernel multi-head attention).

Math (per batch b):
  d2[q,k]   = |q_pos[q] - k_pos[k]|^2   (computed as -d2 via one K=5 augmented matmul)
  s_h[k,q]  = exp(-c_h * d2),  c_h = 1/lengthscale_h^2   (masked keys contribute 0)
  att_h[q,v]= sum_k s_h[k,q] * V[k,h,v] / (sum_k s_h[k,q]*unmasked[k] + 1e-5)
  out[q,o]  = sum_{h,v} att_h[q,v] * w_out[o, h*64+v]

Sharding: 8 cores = (batch b in 0..3) x (query half in 0..1); each core owns
[1024 q, 2048 k]. All inputs host-prepped per core; outputs gathered on host.

Device-side layout is score-transposed: s_h is [k, q] so the attend matmul
(lhsT = values, rhs = scores) contracts k on the PE partition dim directly.
Masking + normalization are folded in: masked V rows are zeroed on the host and
a per-head ones-column (also mask-zeroed) produces the normalizer as psum row 64.
Normalization is deferred past the attend; the reciprocal is computed via
ACT Ln/Exp and broadcast across partitions with a tiny K=8 selection matmul.
Two heads (c=4, c=0.04) are derived from their 4x-smaller neighbors by two DVE
squarings, offloading exp work from the scalar engine.
"""

import numpy as np
from contextlib import ExitStack

B, LQ, LK, DPOS = 4, 2048, 2048, 3
H, V, OUTD = 8, 64, 512
QS = LQ // 2          # q rows per core
KT = LK // 128        # k tiles
V1 = V + 1            # value cols + ones col
NCORES = 8

# head processing order: chain sources immediately before their derived heads
ORDER = [3, 2, 6, 5, 0, 1, 4, 7]
DERIVED = {2: 3, 5: 6}  # derived_head -> source_head, s_d = s_src ** 4

_cache = {}


def _build(cv, use_chains):
    key = (tuple(cv), use_chains)
    if key in _cache:
        return _cache[key]
    import concourse.bacc as bacc
    import concourse.tile as tile
    from concourse import mybir

    f32 = mybir.dt.float32
    bf16 = mybir.dt.bfloat16
    AF = mybir.ActivationFunctionType

    nc = bacc.Bacc("TRN2", target_bir_lowering=False, debug=False,
                   num_devices=NCORES)
    # ka/qa carry a hi/lo bf16 split of the K=5 augmented distance operands:
    # rows [hi(5); lo(5); hi(5)] x [hi(5); hi(5); lo(5)] so the single bf16
    # matmul accumulates hi*hi + lo*hi + hi*lo in fp32 PSUM (lo*lo ~2^-16 is
    # dropped). This avoids fp32 LOW_HIGH double-pass matmuls entirely.
    ka = nc.dram_tensor("ka", [15, LK], bf16, kind="ExternalInput").ap()
    qa = nc.dram_tensor("qa", [15, QS], bf16, kind="ExternalInput").ap()
    vp = nc.dram_tensor("vp", [128, KT, H * V1], bf16, kind="ExternalInput").ap()
    wt = nc.dram_tensor("wt", [128, 4, OUTD], bf16, kind="ExternalInput").ap()
    sel8 = nc.dram_tensor("sel8", [8, 4, 128], bf16, kind="ExternalInput").ap()
    outT = nc.dram_tensor("outT", [OUTD, QS], f32, kind="ExternalOutput").ap()

    with tile.TileContext(nc) as tc, ExitStack() as ctx:
        const = ctx.enter_context(tc.tile_pool(name="const", bufs=1))
        spool = ctx.enter_context(tc.tile_pool(name="spool", bufs=10))
        stage = ctx.enter_context(tc.tile_pool(name="stage", bufs=2))
        obuf = ctx.enter_context(tc.tile_pool(name="obuf", bufs=2))
        psp = ctx.enter_context(tc.tile_pool(name="psum", bufs=3, space="PSUM"))

        ka_sb = const.tile([15, LK], bf16)
        nc.sync.dma_start(out=ka_sb[:], in_=ka)
        qa_sb = const.tile([15, QS], bf16)
        nc.sync.dma_start(out=qa_sb[:], in_=qa)
        vp_sb = const.tile([128, KT, H * V1], bf16)
        nc.sync.dma_start(out=vp_sb[:], in_=vp)
        wt_sb = const.tile([128, 4, OUTD], bf16)
        nc.sync.dma_start(out=wt_sb[:], in_=wt)
        sel8_sb = const.tile([8, 4, 128], bf16)
        nc.sync.dma_start(out=sel8_sb[:], in_=sel8)

        # Phase A: m = -d2 in [k, q] layout, evacuated to bf16 SBUF.
        # m is split into NG group tiles (4 k-tiles each) so per-head exp /
        # squaring / attend pipeline at ~3.7us granularity — PE never idles
        # longer than the HAM MID window, staying at full clock.
        NG, GK = 4, KT // 4
        m_g = [const.tile([128, GK, QS], bf16, tag=f"m{g}", name=f"m{g}")
               for g in range(NG)]
        for kt in range(KT):
            d2 = psp.tile([128, QS], f32, tag="ps")
            for qc in range(2):
                s5 = slice(qc * 512, (qc + 1) * 512)
                nc.tensor.matmul(d2[:, s5],
                                 lhsT=ka_sb[:, kt * 128:(kt + 1) * 128],
                                 rhs=qa_sb[:, s5], start=True, stop=True)
            nc.vector.tensor_copy(out=m_g[kt // GK][:, kt % GK, :], in_=d2[:])

        flat = const.tile([128, 4, QS], bf16)
        norms = const.tile([8, QS], f32)
        s_tiles = {}
        for h in ORDER:
            sg = []
            att = psp.tile([V1, QS], f32, tag="ps")
            for g in range(NG):
                s = spool.tile([128, GK, QS], bf16, tag="s", name=f"s{h}_{g}")
                if use_chains and h in DERIVED:
                    src = s_tiles[DERIVED[h]][g]
                    nc.vector.tensor_mul(s[:], src[:], src[:])
                    nc.vector.tensor_mul(s[:], s[:], s[:])
                else:
                    nc.scalar.activation(out=s[:], in_=m_g[g][:], func=AF.Exp,
                                         scale=float(cv[h]))
                sg.append(s)
                for qc in range(2):
                    s5 = slice(qc * 512, (qc + 1) * 512)
                    for k2 in range(GK):
                        kt = g * GK + k2
                        nc.tensor.matmul(att[:, s5],
                                         lhsT=vp_sb[:, kt, h * V1:(h + 1) * V1],
                                         rhs=s[:, k2, s5],
                                         start=(kt == 0), stop=(kt == KT - 1))
            s_tiles[h] = sg
            r0 = (h % 2) * 64
            nc.vector.tensor_copy(out=flat[r0:r0 + 64, h // 2, :],
                                  in_=att[0:64, :])
            stg = stage.tile([V1, QS], f32, tag="stg")
            nc.vector.tensor_copy(out=stg[64:65, :], in_=att[64:65, :])
            nc.sync.dma_start(out=norms[h:h + 1, :], in_=stg[64:65, :])

        # r = 1/(norm + 1e-5) via exp(-ln(norm + 1e-5)); Ln+Exp share a table set
        eps_t = const.tile([8, 1], f32)
        nc.vector.memset(eps_t[:], 1e-5)
        lnn = const.tile([8, QS], f32)
        nc.scalar.activation(out=lnn[:], in_=norms[:], func=AF.Ln, bias=eps_t[:])
        r_all = const.tile([8, QS], f32)
        nc.scalar.activation(out=r_all[:], in_=lnn[:], func=AF.Exp, scale=-1.0)
        # hi/lo bf16 split of r so the broadcast matmul stays bf16-exact
        r_hi = const.tile([8, QS], bf16)
        nc.vector.tensor_copy(out=r_hi[:], in_=r_all[:])
        r_lo = const.tile([8, QS], bf16)
        nc.vector.tensor_sub(r_lo[:], r_all[:], r_hi[:])

        # broadcast r across partitions (head pair j -> 128 rows) and normalize
        for j in range(4):
            rb = psp.tile([128, QS], f32, tag="ps")
            for qc in range(2):
                s5 = slice(qc * 512, (qc + 1) * 512)
                nc.tensor.matmul(rb[:, s5], lhsT=sel8_sb[:, j, :],
                                 rhs=r_hi[:, s5], start=True, stop=False)
                nc.tensor.matmul(rb[:, s5], lhsT=sel8_sb[:, j, :],
                                 rhs=r_lo[:, s5], start=False, stop=True)
            nc.vector.tensor_mul(flat[:, j, :], flat[:, j, :], rb[:])

        # out projection: outT[o, q] = sum_hv wt[hv, o] * flat[hv, q]
        for ot in range(4):
            po = psp.tile([128, QS], f32, tag="ps")
            for j in range(4):
                for qc in range(2):
                    s5 = slice(qc * 512, (qc + 1) * 512)
                    nc.tensor.matmul(po[:, s5],
                                     lhsT=wt_sb[:, j, ot * 128:(ot + 1) * 128],
                                     rhs=flat[:, j, s5],
                                     start=(j == 0), stop=(j == 3))
            ob = obuf.tile([128, QS], f32, tag="ob")
            nc.scalar.copy(out=ob[:], in_=po[:])
            nc.sync.dma_start(out=outT[ot * 128:(ot + 1) * 128, :], in_=ob[:])

    nc.compile()
    _cache[key] = nc
    return nc


def _hilo(x, bf16):
    hi = x.astype(bf16)
    lo = (x - hi.astype(np.float32)).astype(bf16)
    return hi, lo


def _prep_core(qp, kp, vals, mask, w_out, bf16):
    q2 = (qp * qp).sum(-1)
    one_q = np.ones(QS, np.float32)
    qa5 = np.stack([2 * qp[:, 0], 2 * qp[:, 1], 2 * qp[:, 2], -one_q, -q2]) \
        .astype(np.float32)
    k2 = (kp * kp).sum(-1)
    one_k = np.ones(LK, np.float32)
    ka5 = np.stack([kp[:, 0], kp[:, 1], kp[:, 2], k2, one_k]).astype(np.float32)
    ka_hi, ka_lo = _hilo(ka5, bf16)
    qa_hi, qa_lo = _hilo(qa5, bf16)
    ka = np.concatenate([ka_hi, ka_lo, ka_hi])   # [15, LK]
    qa = np.concatenate([qa_hi, qa_hi, qa_lo])   # [15, QS]
    vv = np.concatenate([vals, np.ones((LK, H, 1), np.float32)], axis=-1)
    vv = vv.copy()
    vv[mask] = 0.0
    vp = vv.reshape(KT, 128, H * V1).transpose(1, 0, 2).astype(bf16)
    wt = np.ascontiguousarray(w_out.T).reshape(4, 128, OUTD) \
        .transpose(1, 0, 2).astype(bf16)
    sel8 = np.zeros((8, 4, 128), np.float32)
    for j in range(4):
        sel8[2 * j, j, :64] = 1.0
        sel8[2 * j + 1, j, 64:] = 1.0
    return {"ka": np.ascontiguousarray(ka), "qa": np.ascontiguousarray(qa),
            "vp": np.ascontiguousarray(vp), "wt": np.ascontiguousarray(wt),
            "sel8": sel8.astype(bf16)}


def kernel(query_positions, key_positions, values, masked_elements,
           lengthscales, w_out, _want_trace=False):
    import ml_dtypes
    from concourse.bass_utils import run_bass_kernel_spmd

    bf16 = ml_dtypes.bfloat16
    qp = np.asarray(query_positions, np.float32)
    kp = np.asarray(key_positions, np.float32)
    vals = np.asarray(values, np.float32)
    mask = np.asarray(masked_elements).astype(bool)
    ls = np.asarray(lengthscales, np.float32)
    w = np.asarray(w_out, np.float32)

    cv = (1.0 / (ls.astype(np.float64) ** 2)).astype(np.float32)
    use_chains = all(
        np.float32(cv[d]) == np.float32(4.0) * np.float32(cv[s])
        for d, s in DERIVED.items())
    nc = _build(tuple(float(x) for x in cv), use_chains)

    in_maps = []
    for c in range(NCORES):
        b, hf = c // 2, c % 2
        in_maps.append(_prep_core(qp[b, hf * QS:(hf + 1) * QS], kp[b],
                                  vals[b], mask[b], w, bf16))
    res = run_bass_kernel_spmd(nc, in_maps, core_ids=list(range(NCORES)),
                               trace=_want_trace)
    out = np.empty((B, LQ, OUTD), np.float32)
    for c in range(NCORES):
        b, hf = c // 2, c % 2
        out[b, hf * QS:(hf + 1) * QS, :] = res.results[c]["outT"].T
    if _want_trace:
        return out, res
    return out
